# revision 31
# baseline (speedup 1.0000x reference)
"""MultiHeadedAttention Trainium2 Bass kernel (v4).

Full inputs in, full output out. 8 cores = 4 batches x 2 head-pairs.

Per-core structure (all matmuls bf16, fp32 PSUM):
  - K/Q projections into per-window tiles k_w[4]/q_w[4] [128, 512]; their
    emission is interleaved into chunk 0's m-loop so attention streams
    behind the input DMA.  bk dropped (cancels in softmax); bq via DVE add.
    Scores scale 1/8 and log2(e) folded into K weights on host: scores
    PSUM holds t = s*log2(e)/8, exp(s/8) = 2^t.
  - V^T tiles vt_w[4] [128, 4, 2, 65]; col 64 = ones so the softmax
    denominator rides along the x-accumulation.  bv applied on host.
  - Attention: 64 global iterations g = (chunk, mb), chunk = 512 n-cols,
    mb = 128 m-rows.  The TensorE runs matmuls strictly in program order,
    so emission is software-pipelined: scores (head-pair row-packed via
    tile_position into one [128, 1024] PSUM pair, 3-deep ring) and exp are
    emitted 2 iterations ahead of the x-accums.  exp alternates ScalarE
    (ACT Exp, scale=ln2) and VectorE (Schraudolph bf16 bit trick:
    int16(round(128*t + B)) bitcast bf16, ~3% max rel err) so both engines
    split the 8.4M-elem softmax.
  - Chunk tails: px [65, 512] PSUM evacuated immediately (ACT h0 / DVE h1)
    to free the single-buffered px banks; sums -> [128, 8] SBUF->SBUF DMA
    -> DVE reciprocal -> DRAM -> partition-broadcast DMA -> r_bc;
    xh = pxe * r_bc on GPSIMD (otherwise idle).  Out-projection per chunk
    (2 accumulated K=64 matmuls per oc) lands in the next chunk boundary;
    PSUM from the scores ring; PSUM->SBUF copy ACT/DVE, DMA out.
Host sums the two per-batch partials and adds bm + wm @ bv in fp32.
"""

import sys

if "/opt/trn_rl_repo" not in sys.path:
    sys.path.insert(0, "/opt/trn_rl_repo")

import numpy as np
import ml_dtypes

BF = ml_dtypes.bfloat16

B, D, N, H = 4, 256, 2048, 4
DIM = D // H  # 64
NW = 4  # 512-wide input windows
MB = 16  # 128-wide m blocks
NC = 4  # 512-wide n chunks
G = NC * MB  # 64 global iterations
LA = 2  # scores/exp emitted this many iterations ahead of x-accums

ALPHA = float(np.log2(np.e) / 8.0)  # folded into wk on host
LN2 = float(np.log(2.0))
C_SCH = 0.0430
TS_SCALE = 128.0
TS_BIAS = 128.0 * (127.0 - C_SCH) + 0.5  # +0.5: truncation -> round

_CACHE = {}


def _emit(ctx, tc, io):
    import concourse.bass as bass
    import concourse.mybir as mybir

    nc = tc.nc
    f32 = mybir.dt.float32
    bf16 = mybir.dt.bfloat16
    i16 = mybir.dt.int16
    EXP = mybir.ActivationFunctionType.Exp
    MUL = mybir.AluOpType.mult
    ADD = mybir.AluOpType.add

    const = ctx.enter_context(tc.tile_pool(name="const", bufs=1))
    xin = ctx.enter_context(tc.tile_pool(name="xin", bufs=4))
    kqp = ctx.enter_context(tc.tile_pool(name="kqp", bufs=4))
    vtp = ctx.enter_context(tc.tile_pool(name="vtp", bufs=4))
    ptp = ctx.enter_context(tc.tile_pool(name="ptp", bufs=4))
    pxe_p = ctx.enter_context(tc.tile_pool(name="pxe", bufs=4))
    xhp = ctx.enter_context(tc.tile_pool(name="xhp", bufs=8))
    work = ctx.enter_context(tc.tile_pool(name="work", bufs=2))
    outp = ctx.enter_context(tc.tile_pool(name="outp", bufs=3))
    psA = ctx.enter_context(tc.tile_pool(name="psA", bufs=3, space="PSUM"))
    psX = ctx.enter_context(tc.tile_pool(name="psX", bufs=2, space="PSUM"))
    dpool = ctx.enter_context(tc.tile_pool(name="dpool", bufs=2, space="DRAM"))

    # ---- constants + inputs: one DMA instruction per tile, spread over
    # the 3 rings in need-order (instruction issue ~0.65us each serializes)
    wu_a = const.tile([128, 128], bf16, tag="wu_a")
    nc.gpsimd.memset(wu_a, 0.0)
    wu_b = const.tile([128, 512], bf16, tag="wu_b")
    nc.gpsimd.memset(wu_b, 0.0)
    junk = const.tile([128, 2], f32, tag="junk")
    nc.scalar.activation(junk[:, 0:1], wu_a[:, 0:1], EXP)  # ACT table load

    wkt_sb = const.tile([128, 2, 128], bf16, tag="wkt")
    nc.sync.dma_start(wkt_sb, io["wkt"].rearrange("(c p) o -> p c o", p=128))
    wqt_sb = const.tile([128, 2, 128], bf16, tag="wqt")
    nc.sync.dma_start(wqt_sb, io["wqt"].rearrange("(c p) o -> p c o", p=128))
    bq_sb = const.tile([128, 1], f32, tag="bq")
    nc.sync.dma_start(bq_sb, io["bq"])
    wvt_sb = const.tile([128, 2, 128], bf16, tag="wvt")
    nc.gpsimd.dma_start(wvt_sb, io["wvt"].rearrange("(c p) o -> p c o", p=128))

    # PE warmup across the input-DMA ramp (HAM clock gate release)
    wu_ps = psA.tile([128, 1024], f32, tag="ps", name="wu_ps")
    for _ in range(8):
        nc.tensor.matmul(wu_ps[:, 0:512], lhsT=wu_a, rhs=wu_b, start=True, stop=True)

    xq_t, xk_t, xv_t = [], [], []
    eng = {"xq": nc.sync, "xk": nc.scalar, "xv": nc.gpsimd}
    for w in range(NW):
        for name, lst in (("xk", xk_t), ("xq", xq_t), ("xv", xv_t)):
            t = xin.tile([128, 2, 512], bf16, tag=name, name=f"{name}{w}")
            src = io[name].rearrange("(c p) n -> p c n", p=128)
            eng[name].dma_start(t, src[:, :, w * 512 : (w + 1) * 512])
            lst.append(t)
    wmt0_sb = const.tile([64, 256], bf16, tag="wmt0")
    nc.gpsimd.dma_start(wmt0_sb, io["wmt0"])
    wmt1_sb = const.tile([64, 256], bf16, tag="wmt1")
    nc.gpsimd.dma_start(wmt1_sb, io["wmt1"])

    # ---- projection / V^T emitters ----
    k_w, q_w, vt_w = [], [], []

    def proj_step(xt, wt, w, bias, lst, nm):
        ps = psA.tile([128, 1024], f32, tag="ps", name=f"ps{nm}{w}")
        nc.tensor.matmul(ps[:, 0:512], lhsT=wt[:, 0, :], rhs=xt[w][:, 0, :], start=True, stop=False)
        nc.tensor.matmul(ps[:, 0:512], lhsT=wt[:, 1, :], rhs=xt[w][:, 1, :], start=False, stop=True)
        dst = kqp.tile([128, 512], bf16, tag=nm, name=f"{nm}{w}")
        if bias is None:
            nc.scalar.copy(dst, ps[:, 0:512])
        else:
            nc.vector.tensor_scalar_add(dst, ps[:, 0:512], bias)
        lst.append(dst)

    def vt_block(w):
        vt = vtp.tile([128, 4, 2, 65], bf16, tag="vt", name=f"vt{w}")
        nc.gpsimd.memset(vt[:, :, :, 64:65], 1.0)
        for off in range(4):
            ms = slice(off * 128, (off + 1) * 128)
            ps = psA.tile([128, 1024], f32, tag="ps", name=f"psvt{w}_{off}")
            pvt = ps[:, 0:128]
            nc.tensor.matmul(pvt, lhsT=xv_t[w][:, 0, ms], rhs=wvt_sb[:, 0, :], start=True, stop=False)
            nc.tensor.matmul(pvt, lhsT=xv_t[w][:, 1, ms], rhs=wvt_sb[:, 1, :], start=False, stop=True)
            nc.vector.tensor_copy(vt[:, off, :, 0:64], pvt.rearrange("m (h d) -> m h d", h=2))
        vt_w.append(vt)

    # ---- software-pipelined attention ----
    sc_t, pt_t, px_t, pxe_t, xh_t = {}, {}, {}, {}, {}
    out_done = []

    def emit_sc(g):
        c, mb = divmod(g, MB)
        w, off = divmod(mb, 4)
        msl = slice(off * 128, (off + 1) * 128)
        sc = psA.tile([128, 1024], f32, tag="ps", name=f"sc{c}_{mb}")
        for h in range(2):
            nc.tensor.matmul(
                sc[:, h * 512 : (h + 1) * 512],
                lhsT=k_w[w][h * 64 : (h + 1) * 64, msl],
                rhs=q_w[c][h * 64 : (h + 1) * 64, :],
                start=True,
                stop=True,
                tile_position=(64 * h, 0),
            )
        sc_t[g] = sc

    def emit_exp(g):
        c, mb = divmod(g, MB)
        sc = sc_t.pop(g)
        pt = ptp.tile([128, 1024], bf16, tag="pt", name="pt")
        if mb % 2 == 1:
            nc.vector.tensor_scalar(pt[:, :].bitcast(i16), sc, TS_SCALE, TS_BIAS, MUL, ADD)
        else:
            nc.scalar.activation(pt, sc, EXP, scale=LN2)
        pt_t[g] = pt

    def emit_sc_exp(g):
        emit_sc(g)
        emit_exp(g)

    def emit_xacc(g):
        c, mb = divmod(g, MB)
        w, off = divmod(mb, 4)
        if mb == 0:
            px_t[c] = [psX.tile([65, 512], f32, tag="px", name=f"px{c}_{h}") for h in range(2)]
        pt = pt_t.pop(g)
        for h in range(2):
            nc.tensor.matmul(
                px_t[c][h],
                lhsT=vt_w[w][:, off, h, :],
                rhs=pt[:, h * 512 : (h + 1) * 512],
                start=(mb == 0),
                stop=(mb == MB - 1),
                skip_group_check=True,
            )

    def out_proj(c):
        for oc in range(2):
            ocs = slice(oc * 128, (oc + 1) * 128)
            po = psA.tile([128, 1024], f32, tag="ps", name=f"po{oc}_{c}")[:, 0:512]
            nc.tensor.matmul(po, lhsT=wmt0_sb[:, ocs], rhs=xh_t[(c, 0)], start=True, stop=False)
            nc.tensor.matmul(po, lhsT=wmt1_sb[:, ocs], rhs=xh_t[(c, 1)], start=False, stop=True)
            ot = outp.tile([128, 512], f32, tag="ot", name="ot")
            if oc == 0:
                nc.scalar.copy(ot, po)
            else:
                nc.vector.tensor_copy(ot, po)
            nc.gpsimd.dma_start(io["out"][ocs, c * 512 : (c + 1) * 512], ot)
        out_done.append(c)

    def emit_evacs(c):
        px = px_t.pop(c)
        pxe = []
        for h in range(2):
            e = pxe_p.tile([65, 512], f32, tag="pxe", name=f"pxe{c}_{h}")
            if h == 0:
                nc.scalar.copy(e, px[h])
            else:
                nc.vector.tensor_copy(e, px[h])
            pxe.append(e)
        return pxe

    def chunk_tail_rest(c, pxe):
        # 1/sums: SBUF->SBUF reshape straight into [128, 8] (h-blocked:
        # h0 -> partitions 0-63, n = 8p+f), cheap reciprocal, DRAM bounce,
        # partition-broadcast read
        s128 = work.tile([128, 8], f32, tag="s128", name=f"s128_{c}")
        for h in range(2):
            nc.sync.dma_start(
                s128[h * 64 : (h + 1) * 64, :], pxe[h][64:65, :], single_packet=True
            )
        r128 = work.tile([128, 8], f32, tag="r128", name=f"r128_{c}")
        nc.vector.reciprocal(r128, s128)
        r_dram = dpool.tile([1, 1024], f32, tag="r_dram", name=f"r_dram{c}")
        nc.sync.dma_start(
            r_dram.rearrange("1 (p f) -> p f", p=128), r128, single_packet=True
        )
        r_bc = work.tile([64, 2, 512], f32, tag="r_bc", name=f"r_bc{c}")
        for h in range(2):
            r_src = bass.AP(
                tensor=r_dram.tensor,
                offset=r_dram.offset + h * 512,
                ap=[[0, 64], [1, 512]],
            )
            nc.sync.dma_start(r_bc[:, h, :], r_src)
        for h in range(2):
            xh = xhp.tile([64, 512], bf16, tag="xh", name=f"xh{c}_{h}")
            if c == NC - 1:
                nc.vector.tensor_mul(xh, pxe[h][0:64, :], r_bc[:, h, :])
            else:
                nc.gpsimd.tensor_mul(xh, pxe[h][0:64, :], r_bc[:, h, :])
            xh_t[(c, h)] = xh
        pxe_t[c] = pxe

    def maybe_proj(ga):
        c, mb = divmod(ga, MB)
        if c == 0 and mb in (4, 8, 12):
            proj_step(xk_t, wkt_sb, mb // 4, None, k_w, "k")
            vt_block(mb // 4)
        # next chunk's q ready well before its first scores
        if mb == 8 and c + 1 <= NC - 1:
            proj_step(xq_t, wqt_sb, c + 1, bq_sb, q_w, "q")

    # prelude: window 0 of everything, then 2 iterations of lookahead
    proj_step(xk_t, wkt_sb, 0, None, k_w, "k")
    proj_step(xq_t, wqt_sb, 0, bq_sb, q_w, "q")
    vt_block(0)
    for g in range(LA):
        emit_sc_exp(g)

    for g in range(G):
        ga = g + LA
        boundary = g % MB == MB - 1
        if ga < G and not boundary:
            maybe_proj(ga)
            emit_sc(ga)
            if ga % MB != 0:
                emit_exp(ga)  # chunk-opening exp deferred past the evacs
        emit_xacc(g)
        if boundary:
            c = g // MB
            pxe = emit_evacs(c)
            if ga < G:
                emit_exp(ga - 1)  # exp of the next chunk's mb 0
                emit_sc_exp(ga)
            chunk_tail_rest(c, pxe)
        elif g % MB == 13 and g // MB >= 1:
            # xh of the previous chunk is ready by now; po runs immediately
            # so its scores-ring slot frees fast
            out_proj(g // MB - 1)
    out_proj(NC - 1)


def _build_nc():
    key = "nc"
    if key in _CACHE:
        return _CACHE[key]
    from contextlib import ExitStack

    import concourse.mybir as mybir
    import concourse.tile as tile
    from concourse import bacc

    f32 = mybir.dt.float32
    bf16 = mybir.dt.bfloat16
    nc = bacc.Bacc("TRN2", target_bir_lowering=False, debug=False, num_devices=8)
    io = {}
    for name, shape, dt_ in (
        ("xq", [256, 2048], bf16),
        ("xk", [256, 2048], bf16),
        ("xv", [256, 2048], bf16),
        ("wqt", [256, 128], bf16),
        ("wkt", [256, 128], bf16),
        ("wvt", [256, 128], bf16),
        ("bq", [128, 1], f32),
        ("wmt0", [64, 256], bf16),
        ("wmt1", [64, 256], bf16),
    ):
        io[name] = nc.dram_tensor(name, shape, dt_, kind="ExternalInput").ap()
    io["out"] = nc.dram_tensor("out", [256, 2048], f32, kind="ExternalOutput").ap()

    with tile.TileContext(nc) as tc:
        with ExitStack() as ctx:
            _emit(ctx, tc, io)
    nc.compile()
    _CACHE[key] = nc
    return nc


def make_in_maps(query, key, value, wq, bq, wk, bk, wv, bv, wm, bm):
    fb = lambda a: np.ascontiguousarray(np.asarray(a, dtype=np.float32)).astype(BF)
    f = lambda a: np.ascontiguousarray(np.asarray(a), dtype=np.float32)
    query, key, value = f(query), f(key), f(value)
    wq, wk, wv, wm = f(wq), f(wk), f(wv), f(wm)
    bq = f(bq)
    in_maps = []
    for c in range(8):
        b, pair = divmod(c, 2)
        hs = (2 * pair, 2 * pair + 1)
        idx = np.array([d * H + h for h in hs for d in range(DIM)])
        m = {
            "xq": fb(query[b]),
            "xk": fb(key[b]),
            "xv": fb(value[b]),
            "wqt": fb(wq[idx].T),
            "wkt": fb(wk[idx].T * ALPHA),
            "wvt": fb(wv[idx].T),
            "bq": f(bq[idx].reshape(128, 1)),
            "wmt0": fb(wm[:, idx[:64]].T),
            "wmt1": fb(wm[:, idx[64:]].T),
        }
        in_maps.append(m)
    return in_maps


def run(in_maps, trace=False, **kw):
    from concourse import bass_utils

    nc = _build_nc()
    return bass_utils.run_bass_kernel_spmd(
        nc, in_maps, core_ids=list(range(8)), trace=trace, **kw
    )


def gather(results, wm, bv, bm):
    wm = np.asarray(wm, dtype=np.float32)
    bv = np.asarray(bv, dtype=np.float32)
    bm = np.asarray(bm, dtype=np.float32)
    corr = bm + wm @ bv
    outs = [np.asarray(r["out"], dtype=np.float32) for r in results]
    return np.stack([outs[2 * b] + outs[2 * b + 1] + corr[:, None] for b in range(B)])


def kernel(query, key, value, wq, bq, wk, bk, wv, bv, wm, bm):
    in_maps = make_in_maps(query, key, value, wq, bq, wk, bk, wv, bv, wm, bm)
    res = run(in_maps)
    return gather(res.results, wm, bv, bm)


# revision 32
# speedup vs baseline: 1.0463x; 1.0463x over previous
"""MultiHeadedAttention Trainium2 Bass kernel (v4).

Full inputs in, full output out. 8 cores = 4 batches x 2 head-pairs.

Per-core structure (all matmuls bf16, fp32 PSUM):
  - K/Q projections into per-window tiles k_w[4]/q_w[4] [128, 512]; their
    emission is interleaved into chunk 0's m-loop so attention streams
    behind the input DMA.  bk dropped (cancels in softmax); bq via DVE add.
    Scores scale 1/8 and log2(e) folded into K weights on host: scores
    PSUM holds t = s*log2(e)/8, exp(s/8) = 2^t.
  - V^T tiles vt_w[4] [128, 4, 2, 65]; col 64 = ones so the softmax
    denominator rides along the x-accumulation.  bv applied on host.
  - Attention: 64 global iterations g = (chunk, mb), chunk = 512 n-cols,
    mb = 128 m-rows.  The TensorE runs matmuls strictly in program order,
    so emission is software-pipelined: scores (head-pair row-packed via
    tile_position into one [128, 1024] PSUM pair, 3-deep ring) and exp are
    emitted 2 iterations ahead of the x-accums.  exp alternates ScalarE
    (ACT Exp, scale=ln2) and VectorE (Schraudolph bf16 bit trick:
    int16(round(128*t + B)) bitcast bf16, ~3% max rel err) so both engines
    split the 8.4M-elem softmax.
  - Chunk tails: px [65, 512] PSUM evacuated immediately (ACT h0 / DVE h1)
    to free the single-buffered px banks; sums -> [128, 8] SBUF->SBUF DMA
    -> DVE reciprocal -> DRAM -> partition-broadcast DMA -> r_bc;
    xh = pxe * r_bc on GPSIMD (otherwise idle).  Out-projection per chunk
    (2 accumulated K=64 matmuls per oc) lands in the next chunk boundary;
    PSUM from the scores ring; PSUM->SBUF copy ACT/DVE, DMA out.
Host sums the two per-batch partials and adds bm + wm @ bv in fp32.
"""

import sys

if "/opt/trn_rl_repo" not in sys.path:
    sys.path.insert(0, "/opt/trn_rl_repo")

import numpy as np
import ml_dtypes

BF = ml_dtypes.bfloat16

B, D, N, H = 4, 256, 2048, 4
DIM = D // H  # 64
NW = 4  # 512-wide input windows
MB = 16  # 128-wide m blocks
NC = 4  # 512-wide n chunks
G = NC * MB  # 64 global iterations
LA = 2  # scores/exp emitted this many iterations ahead of x-accums

ALPHA = float(np.log2(np.e) / 8.0)  # folded into wk on host
LN2 = float(np.log(2.0))
C_SCH = 0.0430
TS_SCALE = 128.0
TS_BIAS = 128.0 * (127.0 - C_SCH) + 0.5  # +0.5: truncation -> round

_CACHE = {}


def _emit(ctx, tc, io):
    import concourse.bass as bass
    import concourse.mybir as mybir

    nc = tc.nc
    f32 = mybir.dt.float32
    bf16 = mybir.dt.bfloat16
    i16 = mybir.dt.int16
    EXP = mybir.ActivationFunctionType.Exp
    MUL = mybir.AluOpType.mult
    ADD = mybir.AluOpType.add

    const = ctx.enter_context(tc.tile_pool(name="const", bufs=1))
    xin = ctx.enter_context(tc.tile_pool(name="xin", bufs=4))
    kqp = ctx.enter_context(tc.tile_pool(name="kqp", bufs=4))
    vtp = ctx.enter_context(tc.tile_pool(name="vtp", bufs=4))
    ptp = ctx.enter_context(tc.tile_pool(name="ptp", bufs=4))
    pxe_p = ctx.enter_context(tc.tile_pool(name="pxe", bufs=4))
    xhp = ctx.enter_context(tc.tile_pool(name="xhp", bufs=8))
    work = ctx.enter_context(tc.tile_pool(name="work", bufs=2))
    outp = ctx.enter_context(tc.tile_pool(name="outp", bufs=3))
    psA = ctx.enter_context(tc.tile_pool(name="psA", bufs=3, space="PSUM"))
    psX = ctx.enter_context(tc.tile_pool(name="psX", bufs=2, space="PSUM"))
    dpool = ctx.enter_context(tc.tile_pool(name="dpool", bufs=2, space="DRAM"))

    # ---- constants + inputs: one DMA instruction per tile, spread over
    # the 3 rings in need-order (instruction issue ~0.65us each serializes)
    wu_a = const.tile([128, 128], bf16, tag="wu_a")
    nc.gpsimd.memset(wu_a, 0.0)
    wu_b = const.tile([128, 512], bf16, tag="wu_b")
    nc.gpsimd.memset(wu_b, 0.0)
    junk = const.tile([128, 2], f32, tag="junk")
    nc.scalar.activation(junk[:, 0:1], wu_a[:, 0:1], EXP)  # ACT table load

    wkt_sb = const.tile([128, 2, 128], bf16, tag="wkt")
    nc.sync.dma_start(wkt_sb, io["wkt"].rearrange("(c p) o -> p c o", p=128))
    wqt_sb = const.tile([128, 2, 128], bf16, tag="wqt")
    nc.sync.dma_start(wqt_sb, io["wqt"].rearrange("(c p) o -> p c o", p=128))
    bq_sb = const.tile([128, 1], f32, tag="bq")
    nc.sync.dma_start(bq_sb, io["bq"])
    wvt_sb = const.tile([128, 2, 128], bf16, tag="wvt")
    nc.gpsimd.dma_start(wvt_sb, io["wvt"].rearrange("(c p) o -> p c o", p=128))

    # PE warmup across the input-DMA ramp (HAM clock gate release)
    wu_ps = psA.tile([128, 1024], f32, tag="ps", name="wu_ps")
    for _ in range(8):
        nc.tensor.matmul(wu_ps[:, 0:512], lhsT=wu_a, rhs=wu_b, start=True, stop=True)

    xq_t, xk_t, xv_t = [], [], []
    eng = {"xq": nc.sync, "xk": nc.scalar, "xv": nc.gpsimd}
    for w in range(NW):
        for name, lst in (("xk", xk_t), ("xq", xq_t), ("xv", xv_t)):
            t = xin.tile([128, 2, 512], bf16, tag=name, name=f"{name}{w}")
            src = io[name].rearrange("(c p) n -> p c n", p=128)
            eng[name].dma_start(t, src[:, :, w * 512 : (w + 1) * 512])
            lst.append(t)
    wmt0_sb = const.tile([64, 256], bf16, tag="wmt0")
    nc.gpsimd.dma_start(wmt0_sb, io["wmt0"])
    wmt1_sb = const.tile([64, 256], bf16, tag="wmt1")
    nc.gpsimd.dma_start(wmt1_sb, io["wmt1"])

    # ---- projection / V^T emitters ----
    k_w, q_w, vt_w = [], [], []

    def proj_step(xt, wt, w, bias, lst, nm):
        ps = psA.tile([128, 1024], f32, tag="ps", name=f"ps{nm}{w}")
        nc.tensor.matmul(ps[:, 0:512], lhsT=wt[:, 0, :], rhs=xt[w][:, 0, :], start=True, stop=False)
        nc.tensor.matmul(ps[:, 0:512], lhsT=wt[:, 1, :], rhs=xt[w][:, 1, :], start=False, stop=True)
        dst = kqp.tile([128, 512], bf16, tag=nm, name=f"{nm}{w}")
        if bias is None:
            nc.scalar.copy(dst, ps[:, 0:512])
        else:
            nc.vector.tensor_scalar_add(dst, ps[:, 0:512], bias)
        lst.append(dst)

    def vt_block(w):
        vt = vtp.tile([128, 4, 2, 65], bf16, tag="vt", name=f"vt{w}")
        nc.gpsimd.memset(vt[:, :, :, 64:65], 1.0)
        for off in range(4):
            ms = slice(off * 128, (off + 1) * 128)
            ps = psA.tile([128, 1024], f32, tag="ps", name=f"psvt{w}_{off}")
            pvt = ps[:, 0:128]
            nc.tensor.matmul(pvt, lhsT=xv_t[w][:, 0, ms], rhs=wvt_sb[:, 0, :], start=True, stop=False)
            nc.tensor.matmul(pvt, lhsT=xv_t[w][:, 1, ms], rhs=wvt_sb[:, 1, :], start=False, stop=True)
            nc.vector.tensor_copy(vt[:, off, :, 0:64], pvt.rearrange("m (h d) -> m h d", h=2))
        vt_w.append(vt)

    # ---- software-pipelined attention ----
    sc_t, pt_t, px_t, pxe_t, xh_t = {}, {}, {}, {}, {}
    out_done = []

    def emit_sc(g):
        c, mb = divmod(g, MB)
        w, off = divmod(mb, 4)
        msl = slice(off * 128, (off + 1) * 128)
        sc = psA.tile([128, 1024], f32, tag="ps", name=f"sc{c}_{mb}")
        for h in range(2):
            nc.tensor.matmul(
                sc[:, h * 512 : (h + 1) * 512],
                lhsT=k_w[w][h * 64 : (h + 1) * 64, msl],
                rhs=q_w[c][h * 64 : (h + 1) * 64, :],
                start=True,
                stop=True,
                tile_position=(64 * h, 0),
            )
        sc_t[g] = sc

    def emit_exp(g):
        c, mb = divmod(g, MB)
        sc = sc_t.pop(g)
        pt = ptp.tile([128, 1024], bf16, tag="pt", name="pt")
        if mb % 2 == 1:
            nc.vector.tensor_scalar(pt[:, :].bitcast(i16), sc, TS_SCALE, TS_BIAS, MUL, ADD)
        else:
            nc.scalar.activation(pt, sc, EXP, scale=LN2)
        pt_t[g] = pt

    def emit_sc_exp(g):
        emit_sc(g)
        emit_exp(g)

    def emit_xacc(g):
        c, mb = divmod(g, MB)
        w, off = divmod(mb, 4)
        if mb == 0:
            px_t[c] = [psX.tile([65, 512], f32, tag="px", name=f"px{c}_{h}") for h in range(2)]
        pt = pt_t.pop(g)
        for h in range(2):
            nc.tensor.matmul(
                px_t[c][h],
                lhsT=vt_w[w][:, off, h, :],
                rhs=pt[:, h * 512 : (h + 1) * 512],
                start=(mb == 0),
                stop=(mb == MB - 1),
                skip_group_check=True,
            )

    def out_proj(c):
        for oc in range(2):
            ocs = slice(oc * 128, (oc + 1) * 128)
            po = psA.tile([128, 1024], f32, tag="ps", name=f"po{oc}_{c}")[:, 0:512]
            nc.tensor.matmul(po, lhsT=wmt0_sb[:, ocs], rhs=xh_t[(c, 0)], start=True, stop=False)
            nc.tensor.matmul(po, lhsT=wmt1_sb[:, ocs], rhs=xh_t[(c, 1)], start=False, stop=True)
            ot = outp.tile([128, 512], f32, tag="ot", name="ot")
            if oc == 0:
                nc.scalar.copy(ot, po)
            else:
                nc.vector.tensor_copy(ot, po)
            nc.gpsimd.dma_start(io["out"][ocs, c * 512 : (c + 1) * 512], ot)
        out_done.append(c)

    def emit_evacs(c):
        px = px_t.pop(c)
        pxe = []
        for h in range(2):
            e = pxe_p.tile([65, 512], f32, tag="pxe", name=f"pxe{c}_{h}")
            if h == 0:
                nc.scalar.copy(e, px[h])
            else:
                nc.vector.tensor_copy(e, px[h])
            pxe.append(e)
        return pxe

    def chunk_tail_rest(c, pxe):
        # 1/sums: SBUF->SBUF reshape straight into [128, 8] (h-blocked:
        # h0 -> partitions 0-63, n = 8p+f), cheap reciprocal, DRAM bounce,
        # partition-broadcast read
        s128 = work.tile([128, 8], f32, tag="s128", name=f"s128_{c}")
        for h in range(2):
            nc.sync.dma_start(
                s128[h * 64 : (h + 1) * 64, :], pxe[h][64:65, :], single_packet=True
            )
        r128 = work.tile([128, 8], f32, tag="r128", name=f"r128_{c}")
        nc.vector.reciprocal(r128, s128)
        r_dram = dpool.tile([1, 1024], f32, tag="r_dram", name=f"r_dram{c}")
        nc.sync.dma_start(
            r_dram.rearrange("1 (p f) -> p f", p=128), r128, single_packet=True
        )
        r_bc = work.tile([64, 2, 512], f32, tag="r_bc", name=f"r_bc{c}")
        for h in range(2):
            r_src = bass.AP(
                tensor=r_dram.tensor,
                offset=r_dram.offset + h * 512,
                ap=[[0, 64], [1, 512]],
            )
            nc.sync.dma_start(r_bc[:, h, :], r_src)
        for h in range(2):
            xh = xhp.tile([64, 512], bf16, tag="xh", name=f"xh{c}_{h}")
            if c == NC - 1:
                nc.vector.tensor_mul(xh, pxe[h][0:64, :], r_bc[:, h, :])
            else:
                nc.gpsimd.tensor_mul(xh, pxe[h][0:64, :], r_bc[:, h, :])
            xh_t[(c, h)] = xh
        pxe_t[c] = pxe

    def maybe_proj(ga):
        c, mb = divmod(ga, MB)
        if c == 0 and mb in (4, 8, 12):
            proj_step(xk_t, wkt_sb, mb // 4, None, k_w, "k")
            vt_block(mb // 4)
        # next chunk's q ready well before its first scores
        if mb == 8 and c + 1 <= NC - 1:
            proj_step(xq_t, wqt_sb, c + 1, bq_sb, q_w, "q")

    # prelude: window 0 of everything, then 2 iterations of lookahead
    proj_step(xk_t, wkt_sb, 0, None, k_w, "k")
    proj_step(xq_t, wqt_sb, 0, bq_sb, q_w, "q")
    vt_block(0)
    for g in range(LA):
        emit_sc_exp(g)

    for g in range(G):
        ga = g + LA
        boundary = g % MB == MB - 1
        if ga < G and not boundary:
            maybe_proj(ga)
            emit_sc_exp(ga)
        emit_xacc(g)
        if boundary:
            c = g // MB
            pxe = emit_evacs(c)
            if ga < G:
                emit_sc_exp(ga)
            chunk_tail_rest(c, pxe)
        elif g % MB == 13 and g // MB >= 1:
            # xh of the previous chunk is ready by now; po runs immediately
            # so its scores-ring slot frees fast
            out_proj(g // MB - 1)
    out_proj(NC - 1)


def _build_nc():
    key = "nc"
    if key in _CACHE:
        return _CACHE[key]
    from contextlib import ExitStack

    import concourse.mybir as mybir
    import concourse.tile as tile
    from concourse import bacc

    f32 = mybir.dt.float32
    bf16 = mybir.dt.bfloat16
    nc = bacc.Bacc("TRN2", target_bir_lowering=False, debug=False, num_devices=8)
    io = {}
    for name, shape, dt_ in (
        ("xq", [256, 2048], bf16),
        ("xk", [256, 2048], bf16),
        ("xv", [256, 2048], bf16),
        ("wqt", [256, 128], bf16),
        ("wkt", [256, 128], bf16),
        ("wvt", [256, 128], bf16),
        ("bq", [128, 1], f32),
        ("wmt0", [64, 256], bf16),
        ("wmt1", [64, 256], bf16),
    ):
        io[name] = nc.dram_tensor(name, shape, dt_, kind="ExternalInput").ap()
    io["out"] = nc.dram_tensor("out", [256, 2048], f32, kind="ExternalOutput").ap()

    with tile.TileContext(nc) as tc:
        with ExitStack() as ctx:
            _emit(ctx, tc, io)
    nc.compile()
    _CACHE[key] = nc
    return nc


def make_in_maps(query, key, value, wq, bq, wk, bk, wv, bv, wm, bm):
    fb = lambda a: np.ascontiguousarray(np.asarray(a, dtype=np.float32)).astype(BF)
    f = lambda a: np.ascontiguousarray(np.asarray(a), dtype=np.float32)
    query, key, value = f(query), f(key), f(value)
    wq, wk, wv, wm = f(wq), f(wk), f(wv), f(wm)
    bq = f(bq)
    in_maps = []
    for c in range(8):
        b, pair = divmod(c, 2)
        hs = (2 * pair, 2 * pair + 1)
        idx = np.array([d * H + h for h in hs for d in range(DIM)])
        m = {
            "xq": fb(query[b]),
            "xk": fb(key[b]),
            "xv": fb(value[b]),
            "wqt": fb(wq[idx].T),
            "wkt": fb(wk[idx].T * ALPHA),
            "wvt": fb(wv[idx].T),
            "bq": f(bq[idx].reshape(128, 1)),
            "wmt0": fb(wm[:, idx[:64]].T),
            "wmt1": fb(wm[:, idx[64:]].T),
        }
        in_maps.append(m)
    return in_maps


def run(in_maps, trace=False, **kw):
    from concourse import bass_utils

    nc = _build_nc()
    return bass_utils.run_bass_kernel_spmd(
        nc, in_maps, core_ids=list(range(8)), trace=trace, **kw
    )


def gather(results, wm, bv, bm):
    wm = np.asarray(wm, dtype=np.float32)
    bv = np.asarray(bv, dtype=np.float32)
    bm = np.asarray(bm, dtype=np.float32)
    corr = bm + wm @ bv
    outs = [np.asarray(r["out"], dtype=np.float32) for r in results]
    return np.stack([outs[2 * b] + outs[2 * b + 1] + corr[:, None] for b in range(B)])


def kernel(query, key, value, wq, bq, wk, bk, wv, bv, wm, bm):
    in_maps = make_in_maps(query, key, value, wq, bq, wk, bk, wv, bv, wm, bm)
    res = run(in_maps)
    return gather(res.results, wm, bv, bm)
